# revision 6
# baseline (speedup 1.0000x reference)
"""GCN 2-layer encoder on 8 Trainium2 NeuronCores (Bass/Tile).

Strategy: dst-node sharding (12500 nodes/core). Aggregation A@X is computed
edge-wise: dma_gather fetches x[src] rows (512B descriptors, 4 SWDGE queues),
the PE contracts 128-edge chunks against DVE-built one-hot-times-norm selector
matrices S into feature-major PSUM windows of 128 dst nodes. Layer order is
aggregate-then-matmul: relu((A@X)@W1+b1), then M=H@W2 is all-gathered and
aggregated the same way for layer 2.
"""
import math
import numpy as np

import concourse.bacc as bacc
import concourse.mybir as mybir
from concourse import tile
from concourse.bass_utils import run_bass_kernel_spmd

N_NODES = 100000
IN_DIM, HID_DIM, OUT_DIM = 128, 128, 64
N_CORES = 8
SHARD = N_NODES // N_CORES          # 12500
WIN = 128                           # dst window (psum cols)
N_WIN = (SHARD + WIN - 1) // WIN    # 98
SHARD_PAD = N_WIN * WIN             # 12544
WG = 3                              # windows per gather group
MAX_CALL_CHUNKS = 8                 # 1024 idxs per dma_gather call
ELEM = 256                          # fp16 elems per table row (512B)
DT16 = mybir.dt.float16
DT32 = mybir.dt.float32


def _plan_layer(dstl_c, rows_c, norm_c, qsize):
    """Per-core edge plan for one layer.

    dstl_c/rows_c/norm_c: per-core lists (len 8) of edge arrays:
      dstl: dst local node id (0..SHARD-1), rows: table row of src, norm: f32.
    qsize: table quarter size (int16 index range).
    Returns dict with uniform (cross-core) structure + per-core data arrays.
    """
    n_cells = N_WIN * 4
    # per-core cell sorting
    per_core = []
    counts = np.zeros((N_CORES, n_cells), np.int64)
    for c in range(N_CORES):
        dstl, rows, norm = dstl_c[c], rows_c[c], norm_c[c]
        win = dstl // WIN
        q = rows // qsize
        wg = win // WG
        order = np.lexsort((win, q, wg))
        dstl, rows, norm, win, q = dstl[order], rows[order], norm[order], win[order], q[order]
        cell = win * 4 + q
        counts[c] = np.bincount(cell, minlength=n_cells)
        per_core.append((dstl, rows, norm, cell))
    # uniform chunks per cell = max over cores
    cell_chunks = np.maximum(np.ceil(counts.max(axis=0) / 128).astype(np.int64), 1)

    # slot layout: cells ordered by (wg, q, w)
    cell_order = []
    n_wg = (N_WIN + WG - 1) // WG
    for g in range(n_wg):
        wins = range(g * WG, min((g + 1) * WG, N_WIN))
        for q in range(4):
            for w in wins:
                cell_order.append(w * 4 + q)
    cell_order = np.array(cell_order)
    chunks_of_cell_in_order = cell_chunks[cell_order]
    cell_chunk_start = np.zeros(n_cells, np.int64)  # first chunk slot of cell
    acc = 0
    for i, cl in enumerate(cell_order):
        cell_chunk_start[cl] = acc
        acc += cell_chunks[cl]
    total_chunks = int(acc)
    total_slots = total_chunks * 128

    # chunk metadata (uniform): window, quarter per chunk slot
    chunk_win = np.zeros(total_chunks, np.int64)
    chunk_q = np.zeros(total_chunks, np.int64)
    for cl in range(n_cells):
        s = cell_chunk_start[cl]
        for k in range(cell_chunks[cl]):
            chunk_win[s + k] = cl // 4
            chunk_q[s + k] = cl % 4

    # calls: consecutive chunks with same quarter, <= MAX_CALL_CHUNKS
    calls = []  # (q, chunk_start, n_chunks)
    i = 0
    while i < total_chunks:
        j = i
        while (j < total_chunks and chunk_q[j] == chunk_q[i]
               and j - i < MAX_CALL_CHUNKS):
            j += 1
        calls.append((int(chunk_q[i]), int(i), int(j - i)))
        i = j

    # per-core data arrays
    idx16 = np.zeros((N_CORES, total_slots), np.int16)
    dn = np.zeros((N_CORES, total_slots), np.float32)
    nm = np.zeros((N_CORES, total_slots), np.float32)
    for c in range(N_CORES):
        dstl, rows, norm, cell = per_core[c]
        # per-cell slices (cells appear contiguously in sorted edge order by
        # (wg, q, win) == cell_order order)
        cnt = counts[c]
        # starting edge offset of each cell in sorted arrays
        edge_off = np.zeros(n_cells, np.int64)
        pos = 0
        for cl in cell_order:
            edge_off[cl] = pos
            pos += cnt[cl]
        for cl in range(n_cells):
            n_e = int(cnt[cl])
            s = int(cell_chunk_start[cl]) * 128
            eo = int(edge_off[cl])
            idx16[c, s:s + n_e] = (rows[eo:eo + n_e] % qsize).astype(np.int16)
            dn[c, s:s + n_e] = (dstl[eo:eo + n_e] % WIN).astype(np.float32)
            nm[c, s:s + n_e] = norm[eo:eo + n_e]
            # pads: idx 0, dn 0, nm 0 (already zeros)

    # pack idxs per call: [128, cols]
    total_cols = total_slots // 16
    idx_packed = np.zeros((N_CORES, 128, total_cols), np.int16)
    for c in range(N_CORES):
        t = idx16[c].reshape(total_slots // 16, 16).T  # [16, cols]
        idx_packed[c] = np.tile(t, (8, 1))
    # dn/nm as [128, chunks]
    dn_t = dn.reshape(N_CORES, total_chunks, 128).transpose(0, 2, 1).copy()
    nm_t = nm.reshape(N_CORES, total_chunks, 128).transpose(0, 2, 1).copy()

    # per-wg structure: list of (win_list, chunk_lo, chunk_hi, call_ids)
    wgs = []
    for g in range(n_wg):
        wins = list(range(g * WG, min((g + 1) * WG, N_WIN)))
        cls = [w * 4 + q for q in range(4) for w in wins]
        lo = min(cell_chunk_start[cl] for cl in cls)
        hi = max(cell_chunk_start[cl] + cell_chunks[cl] for cl in cls)
        call_ids = [i for i, (q, s, n) in enumerate(calls) if lo <= s < hi]
        wgs.append((wins, int(lo), int(hi), call_ids))

    return dict(
        total_chunks=total_chunks, calls=calls, wgs=wgs,
        chunk_win=chunk_win, chunk_q=chunk_q,
        idx_packed=idx_packed, dn=dn_t, nm=nm_t,
        max_wg_chunks=max(hi - lo for (_, lo, hi, _) in wgs),
    )


def _build_plans(edge_index):
    src = np.asarray(edge_index[0], dtype=np.int64)
    dst = np.asarray(edge_index[1], dtype=np.int64)
    loops = np.arange(N_NODES, dtype=np.int64)
    src = np.concatenate([src, loops])
    dst = np.concatenate([dst, loops])
    deg = np.bincount(dst, minlength=N_NODES).astype(np.float64)
    dinv = 1.0 / np.sqrt(deg)
    norm = (dinv[src] * dinv[dst]).astype(np.float32)

    owner = dst // SHARD
    dstl_c, src_c, norm_c = [], [], []
    for c in range(N_CORES):
        m = owner == c
        dstl_c.append((dst[m] - c * SHARD).astype(np.int64))
        src_c.append(src[m])
        norm_c.append(norm[m])

    q1 = (N_NODES + 3) // 4  # 25000
    plan1 = _plan_layer(dstl_c, src_c, norm_c, q1)
    # layer 2: table rows are M rows: 12544*owner(src) + src%12500
    rows2_c = [SHARD_PAD * (s // SHARD) + (s % SHARD) for s in src_c]
    q2 = (SHARD_PAD * N_CORES + 3) // 4  # 25088
    plan2 = _plan_layer(dstl_c, rows2_c, norm_c, q2)
    return plan1, plan2


def _emit_agg_phase(nc, tc, plan, table_d, idx_d, dn_d, nm_d, iota_sb,
                    f_out, out_cb, qsize, tag):
    """Emit gather+aggregate for one layer.

    out_cb(win, psum_ap): consume finished [f_out, WIN] psum window.
    """
    calls = plan["calls"]
    nchunks = plan["total_chunks"]
    with (
        tc.tile_pool(name=f"idx{tag}", bufs=8) as idxp,
        tc.tile_pool(name=f"msg{tag}", bufs=2) as msgp,
        tc.tile_pool(name=f"s{tag}", bufs=8) as sp,
        tc.tile_pool(name=f"dnm{tag}", bufs=1) as dnmp,
        tc.tile_pool(name=f"ps{tag}", bufs=4, space="PSUM") as psp,
    ):
        dn_sb = dnmp.tile([128, nchunks], DT32)
        nc.sync.dma_start(dn_sb[:], dn_d[:])
        nm_sb = dnmp.tile([128, nchunks], DT32)
        nc.sync.dma_start(nm_sb[:], nm_d[:])
        qn = 0
        for (wins, lo, hi, call_ids) in plan["wgs"]:
            nch = hi - lo
            msgs = msgp.tile([128, plan["max_wg_chunks"], ELEM], DT16)
            for ci in call_ids:
                (q, s, n) = calls[ci]
                it = idxp.tile([128, MAX_CALL_CHUNKS * 8], mybir.dt.int16)
                nidx = n * 128
                cols = nidx // 16
                nc.sync.dma_start(it[:, :cols], idx_d[:, s * 8: s * 8 + cols])
                nc.gpsimd.dma_gather(
                    msgs[:, s - lo: s - lo + n, :],
                    table_d[q * qsize: min((q + 1) * qsize, table_d.shape[0]), :],
                    it[:, :cols], nidx, nidx, ELEM, queue_num=qn % 4)
                qn += 1
            # matmuls per chunk
            psums = {}
            first_of_win, last_of_win = {}, {}
            for k in range(lo, hi):
                w = int(plan["chunk_win"][k])
                first_of_win.setdefault(w, k)
                last_of_win[w] = k
            for k in range(lo, hi):
                w = int(plan["chunk_win"][k])
                st = sp.tile([128, WIN], DT16)
                nc.vector.tensor_scalar(
                    out=st[:], in0=iota_sb[:],
                    scalar1=dn_sb[:, k:k + 1], scalar2=nm_sb[:, k:k + 1],
                    op0=mybir.AluOpType.is_equal, op1=mybir.AluOpType.mult)
                if w not in psums:
                    psums[w] = psp.tile([f_out, WIN], DT32, name=f"psw{tag}", tag=f"psw{tag}")
                nc.tensor.matmul(
                    psums[w][:], lhsT=msgs[:, k - lo, 0:f_out], rhs=st[:],
                    start=(k == first_of_win[w]),
                    stop=(k == last_of_win[w]))
            for w in wins:
                out_cb(w, psums[w])


def build_kernel(edge_index, w1, b1, w2, b2, x):
    plan1, plan2 = _build_plans(edge_index)

    nc = bacc.Bacc("TRN2", num_devices=N_CORES, num_swdge_queues=4)
    n1c = plan1["total_chunks"]
    n2c = plan2["total_chunks"]
    xt_d = nc.dram_tensor("xt", [N_NODES, ELEM], DT16, kind="ExternalInput")
    idx1_d = nc.dram_tensor("idx1", [128, n1c * 8], mybir.dt.int16, kind="ExternalInput")
    idx2_d = nc.dram_tensor("idx2", [128, n2c * 8], mybir.dt.int16, kind="ExternalInput")
    dn1_d = nc.dram_tensor("dn1", [128, n1c], DT32, kind="ExternalInput")
    nm1_d = nc.dram_tensor("nm1", [128, n1c], DT32, kind="ExternalInput")
    dn2_d = nc.dram_tensor("dn2", [128, n2c], DT32, kind="ExternalInput")
    nm2_d = nc.dram_tensor("nm2", [128, n2c], DT32, kind="ExternalInput")
    w1_d = nc.dram_tensor("w1", [128, HID_DIM], DT16, kind="ExternalInput")
    w2_d = nc.dram_tensor("w2", [128, OUT_DIM], DT16, kind="ExternalInput")
    b1_d = nc.dram_tensor("b1", [128, 1], DT32, kind="ExternalInput")
    b2_d = nc.dram_tensor("b2", [OUT_DIM, 1], DT32, kind="ExternalInput")
    iota_d = nc.dram_tensor("iota", [128, WIN], DT16, kind="ExternalInput")
    out_d = nc.dram_tensor("outT", [OUT_DIM, SHARD_PAD], DT32, kind="ExternalOutput")
    m_local = nc.dram_tensor("m_local", [SHARD_PAD, ELEM], DT16, kind="Internal")
    m_full = nc.dram_tensor("m_full", [SHARD_PAD * N_CORES, ELEM], DT16,
                            kind="Internal", addr_space="Shared")

    with tile.TileContext(nc) as tc:
        with (
            tc.tile_pool(name="persist", bufs=1) as pp,
            tc.tile_pool(name="mtile", bufs=4) as mp,
            tc.tile_pool(name="dps", bufs=2, space="PSUM") as dps,
        ):
            iota_sb = pp.tile([128, WIN], DT16)
            nc.sync.dma_start(iota_sb[:], iota_d[:])
            w1_sb = pp.tile([128, HID_DIM], DT16)
            nc.sync.dma_start(w1_sb[:], w1_d[:])
            w2_sb = pp.tile([128, OUT_DIM], DT16)
            nc.sync.dma_start(w2_sb[:], w2_d[:])
            b1_sb = pp.tile([128, 1], DT32)
            nc.sync.dma_start(b1_sb[:], b1_d[:])
            b2_sb = pp.tile([OUT_DIM, 1], DT32)
            nc.sync.dma_start(b2_sb[:], b2_d[:])
            aggT = pp.tile([128, SHARD_PAD], DT16)
            hT = pp.tile([128, SHARD_PAD], DT16)

            # ---- layer 1 aggregation: aggT = (A @ X)^T ----
            def l1_out(w, ps):
                nc.vector.tensor_copy(aggT[:, w * WIN:(w + 1) * WIN], ps[:])
            _emit_agg_phase(nc, tc, plan1, xt_d, idx1_d, dn1_d, nm1_d,
                            iota_sb, IN_DIM, l1_out, (N_NODES + 3) // 4, "g1")

            # ---- dense 1: hT = relu(W1^T @ aggT + b1) ----
            for t in range(0, SHARD_PAD, 512):
                wdt = min(512, SHARD_PAD - t)
                ph = dps.tile([128, 512], DT32)
                nc.tensor.matmul(ph[:, :wdt], lhsT=w1_sb[:], rhs=aggT[:, t:t + wdt],
                                 start=True, stop=True)
                nc.scalar.activation(hT[:, t:t + wdt], ph[:, :wdt],
                                     mybir.ActivationFunctionType.Relu,
                                     bias=b1_sb[:, 0:1], scale=1.0)

            # ---- dense 2 + M write: m_local[t] = H_tile @ W2 (padded) ----
            for t in range(N_WIN):
                pm = dps.tile([128, OUT_DIM], DT32)
                nc.tensor.matmul(pm[:], lhsT=hT[:, t * 128:(t + 1) * 128],
                                 rhs=w2_sb[:], start=True, stop=True)
                msb = mp.tile([128, ELEM], DT16)
                nc.vector.tensor_copy(msb[:, 0:OUT_DIM], pm[:])
                nc.vector.memset(msb[:, OUT_DIM:ELEM], 0.0)
                nc.sync.dma_start(m_local[t * 128:(t + 1) * 128, :], msb[:])

            # ---- all-gather M ----
            tc.strict_bb_all_engine_barrier()
            nc.gpsimd.collective_compute(
                "AllGather", mybir.AluOpType.bypass,
                replica_groups=[list(range(N_CORES))],
                ins=[m_local[:]], outs=[m_full[:]])
            tc.strict_bb_all_engine_barrier()

            # ---- layer 2 ----
            n2q = (SHARD_PAD * N_CORES + 3) // 4
            with tc.tile_pool(name="ostage", bufs=4) as osp:
                def l2_out(w, ps):
                    ot = osp.tile([OUT_DIM, WIN], DT32, name="ot")
                    nc.vector.tensor_scalar(
                        out=ot[:], in0=ps[:],
                        scalar1=b2_sb[:, 0:1], scalar2=None,
                        op0=mybir.AluOpType.add)
                    nc.sync.dma_start(out_d[:, w * WIN:(w + 1) * WIN], ot[:])
                _emit_agg_phase(nc, tc, plan2, m_full, idx2_d, dn2_d, nm2_d,
                                iota_sb, OUT_DIM, l2_out, n2q, "g2")
    nc.compile()

    # host-side input tensors
    xt = np.zeros((N_NODES, ELEM), np.float16)
    xt[:, :IN_DIM] = x.astype(np.float16)
    iota = np.tile(np.arange(WIN, dtype=np.float16), (128, 1))
    in_maps = []
    for c in range(N_CORES):
        in_maps.append({
            "xt": xt,
            "idx1": plan1["idx_packed"][c], "idx2": plan2["idx_packed"][c],
            "dn1": plan1["dn"][c], "nm1": plan1["nm"][c],
            "dn2": plan2["dn"][c], "nm2": plan2["nm"][c],
            "w1": w1.astype(np.float16), "w2": w2.astype(np.float16),
            "b1": b1.reshape(-1, 1).astype(np.float32),
            "b2": b2.reshape(-1, 1).astype(np.float32),
            "iota": iota,
        })
    return nc, in_maps


def kernel(x, edge_index, W1, b1, W2, b2):
    x = np.asarray(x); edge_index = np.asarray(edge_index)
    W1 = np.asarray(W1); b1 = np.asarray(b1)
    W2 = np.asarray(W2); b2 = np.asarray(b2)
    nc, in_maps = build_kernel(edge_index, W1, b1, W2, b2, x)
    res = run_bass_kernel_spmd(nc, in_maps, core_ids=list(range(N_CORES)))
    out = np.empty((N_NODES, OUT_DIM), np.float32)
    for c in range(N_CORES):
        out[c * SHARD:(c + 1) * SHARD] = res.results[c]["outT"].T[:SHARD]
    return out


# revision 7
# speedup vs baseline: 29.9451x; 29.9451x over previous
"""GCN 2-layer encoder on 8 Trainium2 NeuronCores (Bass/Tile).

Strategy: dst-node sharding (12500 nodes/core). Aggregation A@X is computed
edge-wise: dma_gather fetches x[src] rows (512B descriptors, 4 SWDGE queues),
the PE contracts 128-edge chunks against DVE-built one-hot-times-norm selector
matrices S into feature-major PSUM windows of 128 dst nodes. Layer order is
aggregate-then-matmul: relu((A@X)@W1+b1), then M=H@W2 is all-gathered and
aggregated the same way for layer 2.
"""
import math
import numpy as np

import concourse.bacc as bacc
import concourse.mybir as mybir
from concourse import tile
from concourse.bass_utils import run_bass_kernel_spmd

N_NODES = 100000
IN_DIM, HID_DIM, OUT_DIM = 128, 128, 64
N_CORES = 8
SHARD = N_NODES // N_CORES          # 12500
WIN = 128                           # dst window (psum cols)
N_WIN = (SHARD + WIN - 1) // WIN    # 98
SHARD_PAD = N_WIN * WIN             # 12544
WG = 3                              # windows per gather group
MAX_CALL_CHUNKS = 8                 # 1024 idxs per dma_gather call
ELEM = 256                          # fp16 elems per table row (512B)
DT16 = mybir.dt.float16
DT32 = mybir.dt.float32


def _plan_layer(dstl_c, rows_c, norm_c, qsize):
    """Per-core edge plan for one layer.

    dstl_c/rows_c/norm_c: per-core lists (len 8) of edge arrays:
      dstl: dst local node id (0..SHARD-1), rows: table row of src, norm: f32.
    qsize: table quarter size (int16 index range).
    Returns dict with uniform (cross-core) structure + per-core data arrays.
    """
    n_cells = N_WIN * 4
    # per-core cell sorting
    per_core = []
    counts = np.zeros((N_CORES, n_cells), np.int64)
    for c in range(N_CORES):
        dstl, rows, norm = dstl_c[c], rows_c[c], norm_c[c]
        win = dstl // WIN
        q = rows // qsize
        wg = win // WG
        order = np.lexsort((win, q, wg))
        dstl, rows, norm, win, q = dstl[order], rows[order], norm[order], win[order], q[order]
        cell = win * 4 + q
        counts[c] = np.bincount(cell, minlength=n_cells)
        per_core.append((dstl, rows, norm, cell))
    # uniform chunks per cell = max over cores
    cell_chunks = np.maximum(np.ceil(counts.max(axis=0) / 128).astype(np.int64), 1)

    # slot layout: cells ordered by (wg, q, w)
    cell_order = []
    n_wg = (N_WIN + WG - 1) // WG
    for g in range(n_wg):
        wins = range(g * WG, min((g + 1) * WG, N_WIN))
        for q in range(4):
            for w in wins:
                cell_order.append(w * 4 + q)
    cell_order = np.array(cell_order)
    chunks_of_cell_in_order = cell_chunks[cell_order]
    cell_chunk_start = np.zeros(n_cells, np.int64)  # first chunk slot of cell
    acc = 0
    for i, cl in enumerate(cell_order):
        cell_chunk_start[cl] = acc
        acc += cell_chunks[cl]
    total_chunks = int(acc)
    total_slots = total_chunks * 128

    # chunk metadata (uniform): window, quarter per chunk slot
    chunk_win = np.zeros(total_chunks, np.int64)
    chunk_q = np.zeros(total_chunks, np.int64)
    for cl in range(n_cells):
        s = cell_chunk_start[cl]
        for k in range(cell_chunks[cl]):
            chunk_win[s + k] = cl // 4
            chunk_q[s + k] = cl % 4

    # calls: consecutive chunks with same quarter, <= MAX_CALL_CHUNKS
    calls = []  # (q, chunk_start, n_chunks)
    i = 0
    while i < total_chunks:
        j = i
        while (j < total_chunks and chunk_q[j] == chunk_q[i]
               and j - i < MAX_CALL_CHUNKS):
            j += 1
        calls.append((int(chunk_q[i]), int(i), int(j - i)))
        i = j

    # per-core data arrays
    idx16 = np.zeros((N_CORES, total_slots), np.int16)
    dn = np.zeros((N_CORES, total_slots), np.float32)
    nm = np.zeros((N_CORES, total_slots), np.float32)
    for c in range(N_CORES):
        dstl, rows, norm, cell = per_core[c]
        # per-cell slices (cells appear contiguously in sorted edge order by
        # (wg, q, win) == cell_order order)
        cnt = counts[c]
        # starting edge offset of each cell in sorted arrays
        edge_off = np.zeros(n_cells, np.int64)
        pos = 0
        for cl in cell_order:
            edge_off[cl] = pos
            pos += cnt[cl]
        for cl in range(n_cells):
            n_e = int(cnt[cl])
            s = int(cell_chunk_start[cl]) * 128
            eo = int(edge_off[cl])
            idx16[c, s:s + n_e] = (rows[eo:eo + n_e] % qsize).astype(np.int16)
            dn[c, s:s + n_e] = (dstl[eo:eo + n_e] % WIN).astype(np.float32)
            nm[c, s:s + n_e] = norm[eo:eo + n_e]
            # pads: idx 0, dn 0, nm 0 (already zeros)

    # pack idxs per call: [128, cols]
    total_cols = total_slots // 16
    idx_packed = np.zeros((N_CORES, 128, total_cols), np.int16)
    for c in range(N_CORES):
        t = idx16[c].reshape(total_slots // 16, 16).T  # [16, cols]
        idx_packed[c] = np.tile(t, (8, 1))
    # dn/nm as [128, chunks]
    dn_t = dn.reshape(N_CORES, total_chunks, 128).transpose(0, 2, 1).copy()
    nm_t = nm.reshape(N_CORES, total_chunks, 128).transpose(0, 2, 1).copy()

    # per-wg structure: list of (win_list, chunk_lo, chunk_hi, call_ids)
    wgs = []
    for g in range(n_wg):
        wins = list(range(g * WG, min((g + 1) * WG, N_WIN)))
        cls = [w * 4 + q for q in range(4) for w in wins]
        lo = min(cell_chunk_start[cl] for cl in cls)
        hi = max(cell_chunk_start[cl] + cell_chunks[cl] for cl in cls)
        call_ids = [i for i, (q, s, n) in enumerate(calls) if lo <= s < hi]
        wgs.append((wins, int(lo), int(hi), call_ids))

    return dict(
        total_chunks=total_chunks, calls=calls, wgs=wgs,
        chunk_win=chunk_win, chunk_q=chunk_q,
        idx_packed=idx_packed, dn=dn_t, nm=nm_t,
        max_wg_chunks=max(hi - lo for (_, lo, hi, _) in wgs),
    )


def _build_plans(edge_index):
    src = np.asarray(edge_index[0], dtype=np.int64)
    dst = np.asarray(edge_index[1], dtype=np.int64)
    loops = np.arange(N_NODES, dtype=np.int64)
    src = np.concatenate([src, loops])
    dst = np.concatenate([dst, loops])
    deg = np.bincount(dst, minlength=N_NODES).astype(np.float64)
    dinv = 1.0 / np.sqrt(deg)
    norm = (dinv[src] * dinv[dst]).astype(np.float32)

    owner = dst // SHARD
    dstl_c, src_c, norm_c = [], [], []
    for c in range(N_CORES):
        m = owner == c
        dstl_c.append((dst[m] - c * SHARD).astype(np.int64))
        src_c.append(src[m])
        norm_c.append(norm[m])

    q1 = (N_NODES + 3) // 4  # 25000
    plan1 = _plan_layer(dstl_c, src_c, norm_c, q1)
    # layer 2: table rows are M rows: 12544*owner(src) + src%12500
    rows2_c = [SHARD_PAD * (s // SHARD) + (s % SHARD) for s in src_c]
    q2 = (SHARD_PAD * N_CORES + 3) // 4  # 25088
    plan2 = _plan_layer(dstl_c, rows2_c, norm_c, q2)
    return plan1, plan2


def _emit_agg_phase(nc, tc, plan, table_d, idx_d, dn_d, nm_d, iota_sb,
                    f_out, out_cb, qsize, tag):
    """Emit gather+aggregate for one layer.

    out_cb(win, psum_ap): consume finished [f_out, WIN] psum window.
    """
    calls = plan["calls"]
    nchunks = plan["total_chunks"]
    with (
        tc.tile_pool(name=f"idx{tag}", bufs=8) as idxp,
        tc.tile_pool(name=f"msg{tag}", bufs=2) as msgp,
        tc.tile_pool(name=f"s{tag}", bufs=8) as sp,
        tc.tile_pool(name=f"dnm{tag}", bufs=1) as dnmp,
        tc.tile_pool(name=f"ps{tag}", bufs=4, space="PSUM") as psp,
    ):
        dn_sb = dnmp.tile([128, nchunks], DT32)
        nc.sync.dma_start(dn_sb[:], dn_d[:])
        nm_sb = dnmp.tile([128, nchunks], DT32)
        nc.sync.dma_start(nm_sb[:], nm_d[:])
        qn = 0
        for (wins, lo, hi, call_ids) in plan["wgs"]:
            nch = hi - lo
            msgs = msgp.tile([128, plan["max_wg_chunks"], ELEM], DT16)
            for ci in call_ids:
                (q, s, n) = calls[ci]
                it = idxp.tile([128, MAX_CALL_CHUNKS * 8], mybir.dt.int16)
                nidx = n * 128
                cols = nidx // 16
                nc.sync.dma_start(it[:, :cols], idx_d[:, s * 8: s * 8 + cols])
                nc.gpsimd.dma_gather(
                    msgs[:, s - lo: s - lo + n, :],
                    table_d[q * qsize: min((q + 1) * qsize, table_d.shape[0]), :],
                    it[:, :cols], nidx, nidx, ELEM, queue_num=qn % 4)
                qn += 1
            # matmuls per chunk
            psums = {}
            first_of_win, last_of_win = {}, {}
            for k in range(lo, hi):
                w = int(plan["chunk_win"][k])
                first_of_win.setdefault(w, k)
                last_of_win[w] = k
            for k in range(lo, hi):
                w = int(plan["chunk_win"][k])
                st = sp.tile([128, WIN], DT16)
                nc.vector.tensor_scalar(
                    out=st[:], in0=iota_sb[:],
                    scalar1=dn_sb[:, k:k + 1], scalar2=nm_sb[:, k:k + 1],
                    op0=mybir.AluOpType.is_equal, op1=mybir.AluOpType.mult)
                if w not in psums:
                    psums[w] = psp.tile([f_out, WIN], DT32, name=f"psw{tag}", tag=f"psw{tag}")
                nc.tensor.matmul(
                    psums[w][:], lhsT=msgs[:, k - lo, 0:f_out], rhs=st[:],
                    start=(k == first_of_win[w]),
                    stop=(k == last_of_win[w]))
            for w in wins:
                out_cb(w, psums[w])


def build_kernel(edge_index, w1, b1, w2, b2, x, reps=1):
    plan1, plan2 = _build_plans(edge_index)

    nc = bacc.Bacc("TRN2", num_devices=N_CORES, num_swdge_queues=4)
    n1c = plan1["total_chunks"]
    n2c = plan2["total_chunks"]
    xt_d = nc.dram_tensor("xt", [N_NODES, ELEM], DT16, kind="ExternalInput")
    idx1_d = nc.dram_tensor("idx1", [128, n1c * 8], mybir.dt.int16, kind="ExternalInput")
    idx2_d = nc.dram_tensor("idx2", [128, n2c * 8], mybir.dt.int16, kind="ExternalInput")
    dn1_d = nc.dram_tensor("dn1", [128, n1c], DT32, kind="ExternalInput")
    nm1_d = nc.dram_tensor("nm1", [128, n1c], DT32, kind="ExternalInput")
    dn2_d = nc.dram_tensor("dn2", [128, n2c], DT32, kind="ExternalInput")
    nm2_d = nc.dram_tensor("nm2", [128, n2c], DT32, kind="ExternalInput")
    w1_d = nc.dram_tensor("w1", [128, HID_DIM], DT16, kind="ExternalInput")
    w2_d = nc.dram_tensor("w2", [128, OUT_DIM], DT16, kind="ExternalInput")
    b1_d = nc.dram_tensor("b1", [128, 1], DT32, kind="ExternalInput")
    b2_d = nc.dram_tensor("b2", [OUT_DIM, 1], DT32, kind="ExternalInput")
    iota_d = nc.dram_tensor("iota", [128, WIN], DT16, kind="ExternalInput")
    out_d = nc.dram_tensor("outT", [OUT_DIM, SHARD_PAD], DT32, kind="ExternalOutput")
    m_local = nc.dram_tensor("m_local", [SHARD_PAD, ELEM], DT16, kind="Internal")
    m_full = nc.dram_tensor("m_full", [SHARD_PAD * N_CORES, ELEM], DT16,
                            kind="Internal", addr_space="Shared")

    with tile.TileContext(nc) as tc:
      for _rep in range(reps):
        with (
            tc.tile_pool(name="persist", bufs=1) as pp,
            tc.tile_pool(name="mtile", bufs=4) as mp,
            tc.tile_pool(name="dps", bufs=2, space="PSUM") as dps,
        ):
            iota_sb = pp.tile([128, WIN], DT16)
            nc.sync.dma_start(iota_sb[:], iota_d[:])
            w1_sb = pp.tile([128, HID_DIM], DT16)
            nc.sync.dma_start(w1_sb[:], w1_d[:])
            w2_sb = pp.tile([128, OUT_DIM], DT16)
            nc.sync.dma_start(w2_sb[:], w2_d[:])
            b1_sb = pp.tile([128, 1], DT32)
            nc.sync.dma_start(b1_sb[:], b1_d[:])
            b2_sb = pp.tile([OUT_DIM, 1], DT32)
            nc.sync.dma_start(b2_sb[:], b2_d[:])
            aggT = pp.tile([128, SHARD_PAD], DT16)
            hT = pp.tile([128, SHARD_PAD], DT16)

            # ---- layer 1 aggregation: aggT = (A @ X)^T ----
            def l1_out(w, ps):
                nc.vector.tensor_copy(aggT[:, w * WIN:(w + 1) * WIN], ps[:])
            _emit_agg_phase(nc, tc, plan1, xt_d, idx1_d, dn1_d, nm1_d,
                            iota_sb, IN_DIM, l1_out, (N_NODES + 3) // 4, "g1")

            # ---- dense 1: hT = relu(W1^T @ aggT + b1) ----
            for t in range(0, SHARD_PAD, 512):
                wdt = min(512, SHARD_PAD - t)
                ph = dps.tile([128, 512], DT32)
                nc.tensor.matmul(ph[:, :wdt], lhsT=w1_sb[:], rhs=aggT[:, t:t + wdt],
                                 start=True, stop=True)
                nc.scalar.activation(hT[:, t:t + wdt], ph[:, :wdt],
                                     mybir.ActivationFunctionType.Relu,
                                     bias=b1_sb[:, 0:1], scale=1.0)

            # ---- dense 2 + M write: m_local[t] = H_tile @ W2 (padded) ----
            for t in range(N_WIN):
                pm = dps.tile([128, OUT_DIM], DT32)
                nc.tensor.matmul(pm[:], lhsT=hT[:, t * 128:(t + 1) * 128],
                                 rhs=w2_sb[:], start=True, stop=True)
                msb = mp.tile([128, ELEM], DT16)
                nc.vector.tensor_copy(msb[:, 0:OUT_DIM], pm[:])
                nc.vector.memset(msb[:, OUT_DIM:ELEM], 0.0)
                nc.sync.dma_start(m_local[t * 128:(t + 1) * 128, :], msb[:])

            # ---- all-gather M ----
            tc.strict_bb_all_engine_barrier()
            nc.gpsimd.collective_compute(
                "AllGather", mybir.AluOpType.bypass,
                replica_groups=[list(range(N_CORES))],
                ins=[m_local[:]], outs=[m_full[:]])
            tc.strict_bb_all_engine_barrier()

            # ---- layer 2 ----
            n2q = (SHARD_PAD * N_CORES + 3) // 4
            with tc.tile_pool(name="ostage", bufs=4) as osp:
                def l2_out(w, ps):
                    ot = osp.tile([OUT_DIM, WIN], DT32, name="ot")
                    nc.vector.tensor_scalar(
                        out=ot[:], in0=ps[:],
                        scalar1=b2_sb[:, 0:1], scalar2=None,
                        op0=mybir.AluOpType.add)
                    nc.sync.dma_start(out_d[:, w * WIN:(w + 1) * WIN], ot[:])
                _emit_agg_phase(nc, tc, plan2, m_full, idx2_d, dn2_d, nm2_d,
                                iota_sb, OUT_DIM, l2_out, n2q, "g2")
    nc.compile()

    # host-side input tensors
    xt = np.zeros((N_NODES, ELEM), np.float16)
    xt[:, :IN_DIM] = x.astype(np.float16)
    iota = np.tile(np.arange(WIN, dtype=np.float16), (128, 1))
    in_maps = []
    for c in range(N_CORES):
        in_maps.append({
            "xt": xt,
            "idx1": plan1["idx_packed"][c], "idx2": plan2["idx_packed"][c],
            "dn1": plan1["dn"][c], "nm1": plan1["nm"][c],
            "dn2": plan2["dn"][c], "nm2": plan2["nm"][c],
            "w1": w1.astype(np.float16), "w2": w2.astype(np.float16),
            "b1": b1.reshape(-1, 1).astype(np.float32),
            "b2": b2.reshape(-1, 1).astype(np.float32),
            "iota": iota,
        })
    return nc, in_maps


def kernel(x, edge_index, W1, b1, W2, b2):
    x = np.asarray(x); edge_index = np.asarray(edge_index)
    W1 = np.asarray(W1); b1 = np.asarray(b1)
    W2 = np.asarray(W2); b2 = np.asarray(b2)
    nc, in_maps = build_kernel(edge_index, W1, b1, W2, b2, x)
    res = run_bass_kernel_spmd(nc, in_maps, core_ids=list(range(N_CORES)))
    out = np.empty((N_NODES, OUT_DIM), np.float32)
    for c in range(N_CORES):
        out[c * SHARD:(c + 1) * SHARD] = res.results[c]["outT"].T[:SHARD]
    return out


# revision 8
# speedup vs baseline: 35.1052x; 1.1723x over previous
"""GCN 2-layer encoder on 8 Trainium2 NeuronCores (Bass/Tile).

Strategy: dst-node sharding (12500 nodes/core). Aggregation A@X is computed
edge-wise: dma_gather fetches x[src] rows (512B descriptors, 4 SWDGE queues),
the PE contracts 128-edge chunks against DVE-built one-hot-times-norm selector
matrices S into feature-major PSUM windows of 128 dst nodes. Layer order is
aggregate-then-matmul: relu((A@X)@W1+b1), then M=H@W2 is all-gathered and
aggregated the same way for layer 2.
"""
import math
import numpy as np

import concourse.bacc as bacc
import concourse.mybir as mybir
from concourse import tile
from concourse.bass_utils import run_bass_kernel_spmd

N_NODES = 100000
IN_DIM, HID_DIM, OUT_DIM = 128, 128, 64
N_CORES = 8
SHARD = N_NODES // N_CORES          # 12500
WIN = 128                           # dst window (psum cols)
N_WIN = (SHARD + WIN - 1) // WIN    # 98
SHARD_PAD = N_WIN * WIN             # 12544
WG = 3                              # windows per gather group
MAX_CALL_CHUNKS = 8                 # 1024 idxs per dma_gather call
ELEM = 256                          # fp16 elems per table row (512B)
DT16 = mybir.dt.float16
DT32 = mybir.dt.float32


def _plan_layer(dstl_c, rows_c, norm_c, qsize):
    """Per-core edge plan for one layer.

    dstl_c/rows_c/norm_c: per-core lists (len 8) of edge arrays:
      dstl: dst local node id (0..SHARD-1), rows: table row of src, norm: f32.
    qsize: table quarter size (int16 index range).
    Returns dict with uniform (cross-core) structure + per-core data arrays.
    """
    n_cells = N_WIN * 4
    # per-core cell sorting
    per_core = []
    counts = np.zeros((N_CORES, n_cells), np.int64)
    for c in range(N_CORES):
        dstl, rows, norm = dstl_c[c], rows_c[c], norm_c[c]
        win = dstl // WIN
        q = rows // qsize
        wg = win // WG
        order = np.lexsort((win, q, wg))
        dstl, rows, norm, win, q = dstl[order], rows[order], norm[order], win[order], q[order]
        cell = win * 4 + q
        counts[c] = np.bincount(cell, minlength=n_cells)
        per_core.append((dstl, rows, norm, cell))
    # uniform chunks per cell = max over cores
    cell_chunks = np.maximum(np.ceil(counts.max(axis=0) / 128).astype(np.int64), 1)

    # slot layout: cells ordered by (wg, q, w)
    cell_order = []
    n_wg = (N_WIN + WG - 1) // WG
    for g in range(n_wg):
        wins = range(g * WG, min((g + 1) * WG, N_WIN))
        for q in range(4):
            for w in wins:
                cell_order.append(w * 4 + q)
    cell_order = np.array(cell_order)
    chunks_of_cell_in_order = cell_chunks[cell_order]
    cell_chunk_start = np.zeros(n_cells, np.int64)  # first chunk slot of cell
    acc = 0
    for i, cl in enumerate(cell_order):
        cell_chunk_start[cl] = acc
        acc += cell_chunks[cl]
    total_chunks = int(acc)
    total_slots = total_chunks * 128

    # chunk metadata (uniform): window, quarter per chunk slot
    chunk_win = np.zeros(total_chunks, np.int64)
    chunk_q = np.zeros(total_chunks, np.int64)
    for cl in range(n_cells):
        s = cell_chunk_start[cl]
        for k in range(cell_chunks[cl]):
            chunk_win[s + k] = cl // 4
            chunk_q[s + k] = cl % 4

    # calls: consecutive chunks with same quarter, <= MAX_CALL_CHUNKS
    calls = []  # (q, chunk_start, n_chunks)
    i = 0
    while i < total_chunks:
        j = i
        while (j < total_chunks and chunk_q[j] == chunk_q[i]
               and j - i < MAX_CALL_CHUNKS):
            j += 1
        calls.append((int(chunk_q[i]), int(i), int(j - i)))
        i = j

    # per-core data arrays
    idx16 = np.zeros((N_CORES, total_slots), np.int16)
    dn = np.zeros((N_CORES, total_slots), np.float32)
    nm = np.zeros((N_CORES, total_slots), np.float32)
    for c in range(N_CORES):
        dstl, rows, norm, cell = per_core[c]
        # per-cell slices (cells appear contiguously in sorted edge order by
        # (wg, q, win) == cell_order order)
        cnt = counts[c]
        # starting edge offset of each cell in sorted arrays
        edge_off = np.zeros(n_cells, np.int64)
        pos = 0
        for cl in cell_order:
            edge_off[cl] = pos
            pos += cnt[cl]
        for cl in range(n_cells):
            n_e = int(cnt[cl])
            s = int(cell_chunk_start[cl]) * 128
            eo = int(edge_off[cl])
            idx16[c, s:s + n_e] = (rows[eo:eo + n_e] % qsize).astype(np.int16)
            dn[c, s:s + n_e] = (dstl[eo:eo + n_e] % WIN).astype(np.float32)
            nm[c, s:s + n_e] = norm[eo:eo + n_e]
            # pads: idx 0, dn 0, nm 0 (already zeros)

    # pack idxs per call: [128, cols]
    total_cols = total_slots // 16
    idx_packed = np.zeros((N_CORES, 128, total_cols), np.int16)
    for c in range(N_CORES):
        t = idx16[c].reshape(total_slots // 16, 16).T  # [16, cols]
        idx_packed[c] = np.tile(t, (8, 1))
    # dn/nm as [128, chunks]
    dn_t = dn.reshape(N_CORES, total_chunks, 128).transpose(0, 2, 1).copy()
    nm_t = nm.reshape(N_CORES, total_chunks, 128).transpose(0, 2, 1).copy()

    # per-wg structure: list of (win_list, chunk_lo, chunk_hi, call_ids)
    wgs = []
    for g in range(n_wg):
        wins = list(range(g * WG, min((g + 1) * WG, N_WIN)))
        cls = [w * 4 + q for q in range(4) for w in wins]
        lo = min(cell_chunk_start[cl] for cl in cls)
        hi = max(cell_chunk_start[cl] + cell_chunks[cl] for cl in cls)
        call_ids = [i for i, (q, s, n) in enumerate(calls) if lo <= s < hi]
        wgs.append((wins, int(lo), int(hi), call_ids))

    return dict(
        total_chunks=total_chunks, calls=calls, wgs=wgs,
        chunk_win=chunk_win, chunk_q=chunk_q,
        idx_packed=idx_packed, dn=dn_t, nm=nm_t,
        max_wg_chunks=max(hi - lo for (_, lo, hi, _) in wgs),
    )


def _build_plans(edge_index):
    src = np.asarray(edge_index[0], dtype=np.int64)
    dst = np.asarray(edge_index[1], dtype=np.int64)
    loops = np.arange(N_NODES, dtype=np.int64)
    src = np.concatenate([src, loops])
    dst = np.concatenate([dst, loops])
    deg = np.bincount(dst, minlength=N_NODES).astype(np.float64)
    dinv = 1.0 / np.sqrt(deg)
    norm = (dinv[src] * dinv[dst]).astype(np.float32)

    owner = dst // SHARD
    dstl_c, src_c, norm_c = [], [], []
    for c in range(N_CORES):
        m = owner == c
        dstl_c.append((dst[m] - c * SHARD).astype(np.int64))
        src_c.append(src[m])
        norm_c.append(norm[m])

    q1 = (N_NODES + 3) // 4  # 25000
    plan1 = _plan_layer(dstl_c, src_c, norm_c, q1)
    # layer 2: table rows are M rows: 12544*owner(src) + src%12500
    rows2_c = [SHARD_PAD * (s // SHARD) + (s % SHARD) for s in src_c]
    q2 = (SHARD_PAD * N_CORES + 3) // 4  # 25088
    plan2 = _plan_layer(dstl_c, rows2_c, norm_c, q2)
    return plan1, plan2


def _emit_agg_phase(nc, tc, plan, table_d, idx_d, dn_d, nm_d, iota_sb,
                    f_out, out_cb, qsize, tag):
    """Emit gather+aggregate for one layer.

    out_cb(win, psum_ap): consume finished [f_out, WIN] psum window.
    """
    calls = plan["calls"]
    nchunks = plan["total_chunks"]
    with (
        tc.tile_pool(name=f"idx{tag}", bufs=3) as idxp,
        tc.tile_pool(name=f"msg{tag}", bufs=2) as msgp,
        tc.tile_pool(name=f"s{tag}", bufs=8) as sp,
        tc.tile_pool(name=f"dnm{tag}", bufs=1) as dnmp,
        tc.tile_pool(name=f"ps{tag}", bufs=4, space="PSUM") as psp,
    ):
        dn_sb = dnmp.tile([128, nchunks], DT32)
        nc.sync.dma_start(dn_sb[:], dn_d[:])
        nm_sb = dnmp.tile([128, nchunks], DT32)
        nc.sync.dma_start(nm_sb[:], nm_d[:])
        qn = 0
        for (wins, lo, hi, call_ids) in plan["wgs"]:
            nch = hi - lo
            msgs = msgp.tile([128, plan["max_wg_chunks"], ELEM], DT16)
            it = idxp.tile([128, plan["max_wg_chunks"] * 8], mybir.dt.int16)
            nc.sync.dma_start(it[:, :nch * 8], idx_d[:, lo * 8: hi * 8])
            for ci in call_ids:
                (q, s, n) = calls[ci]
                nidx = n * 128
                cols = nidx // 16
                nc.gpsimd.dma_gather(
                    msgs[:, s - lo: s - lo + n, :],
                    table_d[q * qsize: min((q + 1) * qsize, table_d.shape[0]), :],
                    it[:, (s - lo) * 8: (s - lo) * 8 + cols],
                    nidx, nidx, ELEM, queue_num=qn % 4)
                qn += 1
            # matmuls per chunk
            psums = {}
            first_of_win, last_of_win = {}, {}
            for k in range(lo, hi):
                w = int(plan["chunk_win"][k])
                first_of_win.setdefault(w, k)
                last_of_win[w] = k
            for k in range(lo, hi):
                w = int(plan["chunk_win"][k])
                st = sp.tile([128, WIN], DT16)
                nc.vector.tensor_scalar(
                    out=st[:], in0=iota_sb[:],
                    scalar1=dn_sb[:, k:k + 1], scalar2=nm_sb[:, k:k + 1],
                    op0=mybir.AluOpType.is_equal, op1=mybir.AluOpType.mult)
                if w not in psums:
                    psums[w] = psp.tile([f_out, WIN], DT32, name=f"psw{tag}", tag=f"psw{tag}")
                nc.tensor.matmul(
                    psums[w][:], lhsT=msgs[:, k - lo, 0:f_out], rhs=st[:],
                    start=(k == first_of_win[w]),
                    stop=(k == last_of_win[w]))
            for w in wins:
                out_cb(w, psums[w])


def build_kernel(edge_index, w1, b1, w2, b2, x, reps=1):
    plan1, plan2 = _build_plans(edge_index)

    nc = bacc.Bacc("TRN2", num_devices=N_CORES, num_swdge_queues=4)
    n1c = plan1["total_chunks"]
    n2c = plan2["total_chunks"]
    xt_d = nc.dram_tensor("xt", [N_NODES, ELEM], DT16, kind="ExternalInput")
    idx1_d = nc.dram_tensor("idx1", [128, n1c * 8], mybir.dt.int16, kind="ExternalInput")
    idx2_d = nc.dram_tensor("idx2", [128, n2c * 8], mybir.dt.int16, kind="ExternalInput")
    dn1_d = nc.dram_tensor("dn1", [128, n1c], DT32, kind="ExternalInput")
    nm1_d = nc.dram_tensor("nm1", [128, n1c], DT32, kind="ExternalInput")
    dn2_d = nc.dram_tensor("dn2", [128, n2c], DT32, kind="ExternalInput")
    nm2_d = nc.dram_tensor("nm2", [128, n2c], DT32, kind="ExternalInput")
    w1_d = nc.dram_tensor("w1", [128, HID_DIM], DT16, kind="ExternalInput")
    w2_d = nc.dram_tensor("w2", [128, OUT_DIM], DT16, kind="ExternalInput")
    b1_d = nc.dram_tensor("b1", [128, 1], DT32, kind="ExternalInput")
    b2_d = nc.dram_tensor("b2", [OUT_DIM, 1], DT32, kind="ExternalInput")
    iota_d = nc.dram_tensor("iota", [128, WIN], DT16, kind="ExternalInput")
    out_d = nc.dram_tensor("outT", [OUT_DIM, SHARD_PAD], DT32, kind="ExternalOutput")
    m_local = nc.dram_tensor("m_local", [SHARD_PAD, ELEM], DT16, kind="Internal")
    m_full = nc.dram_tensor("m_full", [SHARD_PAD * N_CORES, ELEM], DT16,
                            kind="Internal", addr_space="Shared")

    with tile.TileContext(nc) as tc:
      for _rep in range(reps):
        with (
            tc.tile_pool(name="persist", bufs=1) as pp,
            tc.tile_pool(name="mtile", bufs=4) as mp,
            tc.tile_pool(name="dps", bufs=2, space="PSUM") as dps,
        ):
            iota_sb = pp.tile([128, WIN], DT16)
            nc.sync.dma_start(iota_sb[:], iota_d[:])
            w1_sb = pp.tile([128, HID_DIM], DT16)
            nc.sync.dma_start(w1_sb[:], w1_d[:])
            w2_sb = pp.tile([128, OUT_DIM], DT16)
            nc.sync.dma_start(w2_sb[:], w2_d[:])
            b1_sb = pp.tile([128, 1], DT32)
            nc.sync.dma_start(b1_sb[:], b1_d[:])
            b2_sb = pp.tile([OUT_DIM, 1], DT32)
            nc.sync.dma_start(b2_sb[:], b2_d[:])
            aggT = pp.tile([128, SHARD_PAD], DT16)
            hT = pp.tile([128, SHARD_PAD], DT16)

            # ---- layer 1 aggregation: aggT = (A @ X)^T ----
            def l1_out(w, ps):
                nc.vector.tensor_copy(aggT[:, w * WIN:(w + 1) * WIN], ps[:])
            _emit_agg_phase(nc, tc, plan1, xt_d, idx1_d, dn1_d, nm1_d,
                            iota_sb, IN_DIM, l1_out, (N_NODES + 3) // 4, "g1")

            # ---- dense 1: hT = relu(W1^T @ aggT + b1) ----
            for t in range(0, SHARD_PAD, 512):
                wdt = min(512, SHARD_PAD - t)
                ph = dps.tile([128, 512], DT32)
                nc.tensor.matmul(ph[:, :wdt], lhsT=w1_sb[:], rhs=aggT[:, t:t + wdt],
                                 start=True, stop=True)
                nc.scalar.activation(hT[:, t:t + wdt], ph[:, :wdt],
                                     mybir.ActivationFunctionType.Relu,
                                     bias=b1_sb[:, 0:1], scale=1.0)

            # ---- dense 2 + M write: m_local[t] = H_tile @ W2 (padded) ----
            for t in range(N_WIN):
                pm = dps.tile([128, OUT_DIM], DT32)
                nc.tensor.matmul(pm[:], lhsT=hT[:, t * 128:(t + 1) * 128],
                                 rhs=w2_sb[:], start=True, stop=True)
                msb = mp.tile([128, ELEM], DT16)
                nc.vector.tensor_copy(msb[:, 0:OUT_DIM], pm[:])
                nc.vector.memset(msb[:, OUT_DIM:ELEM], 0.0)
                nc.sync.dma_start(m_local[t * 128:(t + 1) * 128, :], msb[:])

            # ---- all-gather M ----
            tc.strict_bb_all_engine_barrier()
            nc.gpsimd.collective_compute(
                "AllGather", mybir.AluOpType.bypass,
                replica_groups=[list(range(N_CORES))],
                ins=[m_local[:]], outs=[m_full[:]])
            tc.strict_bb_all_engine_barrier()

            # ---- layer 2 ----
            n2q = (SHARD_PAD * N_CORES + 3) // 4
            with tc.tile_pool(name="ostage", bufs=4) as osp:
                def l2_out(w, ps):
                    ot = osp.tile([OUT_DIM, WIN], DT32, name="ot")
                    nc.vector.tensor_scalar(
                        out=ot[:], in0=ps[:],
                        scalar1=b2_sb[:, 0:1], scalar2=None,
                        op0=mybir.AluOpType.add)
                    nc.sync.dma_start(out_d[:, w * WIN:(w + 1) * WIN], ot[:])
                _emit_agg_phase(nc, tc, plan2, m_full, idx2_d, dn2_d, nm2_d,
                                iota_sb, OUT_DIM, l2_out, n2q, "g2")
    nc.compile()

    # host-side input tensors
    xt = np.zeros((N_NODES, ELEM), np.float16)
    xt[:, :IN_DIM] = x.astype(np.float16)
    iota = np.tile(np.arange(WIN, dtype=np.float16), (128, 1))
    in_maps = []
    for c in range(N_CORES):
        in_maps.append({
            "xt": xt,
            "idx1": plan1["idx_packed"][c], "idx2": plan2["idx_packed"][c],
            "dn1": plan1["dn"][c], "nm1": plan1["nm"][c],
            "dn2": plan2["dn"][c], "nm2": plan2["nm"][c],
            "w1": w1.astype(np.float16), "w2": w2.astype(np.float16),
            "b1": b1.reshape(-1, 1).astype(np.float32),
            "b2": b2.reshape(-1, 1).astype(np.float32),
            "iota": iota,
        })
    return nc, in_maps


def kernel(x, edge_index, W1, b1, W2, b2):
    x = np.asarray(x); edge_index = np.asarray(edge_index)
    W1 = np.asarray(W1); b1 = np.asarray(b1)
    W2 = np.asarray(W2); b2 = np.asarray(b2)
    nc, in_maps = build_kernel(edge_index, W1, b1, W2, b2, x)
    res = run_bass_kernel_spmd(nc, in_maps, core_ids=list(range(N_CORES)))
    out = np.empty((N_NODES, OUT_DIM), np.float32)
    for c in range(N_CORES):
        out[c * SHARD:(c + 1) * SHARD] = res.results[c]["outT"].T[:SHARD]
    return out
